# revision 32
# baseline (speedup 1.0000x reference)
"""Spectral pooling (FFT2 -> crop low freqs -> IFFT2) as dense DFT matmuls on TRN2.

Input  x: (32, 256, 64, 64) fp32  -- channels 0:128 real part, 128:256 imag part
Output y: (32, 256, 32, 32) fp32

Math: per complex image X (64x64), Y = A @ X @ A.T with
  A = sqrt(1/(64*32)) * IDFT32 @ Crop @ DFT64   (32x64 complex)
Sharding: batch dim across 8 cores (4 batches/core), no communication.

On-chip scheme (bf16 matmuls, K=M=128, data stationary, DFT matrices moving):
  stage 1: lhsT = [[Xr_c, Xr_c2], [Xr_c1, Xr_c3]] block matrix (a "quad" of 4
           complex channels), rhs = block-structured [Ar.T/Ai.T] constants
           -> psum1 = [Pr_c|Pr_c1|Pi_c|Pi_c1 ; Pr_c2|Pr_c3|Pi_c2|Pi_c3].T-ish
  stage 2: lhsT = psum1 column halves, rhs = block-diag constants -> Y quad
  2 matmuls per stage per quad (real+imag accumulate in PSUM).
  fp32->bf16 input cast happens inside the SWDGE load DMA; outputs restored to
  fp32 by the PSUM->SBUF copies (DVE stage-1 cast, ACT stage-2).
"""

import math

import numpy as np

from concourse import bass, mybir
from concourse.bass_utils import run_bass_kernel_spmd
from concourse.tile import TileContext

N_CORES = 8
B_FULL, C2, H, W = 32, 256, 64, 64
HP, WP = 32, 32
BPC = B_FULL // N_CORES  # batches per core

F32 = mybir.dt.float32
BF16 = mybir.dt.bfloat16


def _split_multi_waits(nc):
    """This walrus build rejects instructions carrying more than one semaphore
    wait. Hoist extra waits onto same-engine NOPs inserted just before the
    instruction (engine queues execute in order, so blocking is equivalent)."""
    n_split = 0
    for f in nc.m.functions:
        for bb in f.blocks:
            insts = bb.instructions
            out = []
            for inst in insts:
                si = inst.sync_info
                waits = list(si.on_wait) if si and si.on_wait else []
                if len(waits) > 1:
                    si.on_wait = waits[-1:]
                    for w in waits[:-1]:
                        nop = mybir.InstNoOp(
                            name=nc.get_next_instruction_name(),
                            ins=[],
                            outs=[],
                            engine=inst.engine,
                            sync_info=mybir.SyncInfo(on_wait=[w], on_update=[]),
                        )
                        out.append(nop)
                        n_split += 1
                out.append(inst)
            if len(out) != len(insts):
                insts[:] = out
    return n_split


def _dft_constants():
    """[4, 128, 128] fp32: stage-1 (D1r, D1i) and stage-2 (D2r, D2i) moving
    operands."""
    topf = int(math.ceil(H * 0.5 / 2))  # 16
    midf = H // 2 + topf  # 48
    F = np.exp(-2j * np.pi * np.outer(np.arange(H), np.arange(H)) / H)
    G = np.exp(2j * np.pi * np.outer(np.arange(HP), np.arange(HP)) / HP)
    keep = list(range(topf)) + list(range(midf, H))
    S = np.zeros((HP, H))
    S[np.arange(HP), keep] = 1
    A = (G @ S @ F) / np.sqrt(H * W * HP * WP) ** 0.5
    ArT = A.real.astype(np.float32).T  # [64, 32]
    AiT = A.imag.astype(np.float32).T

    D1r = np.zeros((128, 128), np.float32)
    D1i = np.zeros((128, 128), np.float32)
    D1r[:64, 0:32] = ArT
    D1r[64:, 32:64] = ArT
    D1r[:64, 64:96] = AiT
    D1r[64:, 96:128] = AiT
    D1i[:64, 0:32] = -AiT
    D1i[64:, 32:64] = -AiT
    D1i[:64, 64:96] = ArT
    D1i[64:, 96:128] = ArT

    C2r = np.concatenate([ArT, AiT], axis=1)  # [64, 64]
    C2i = np.concatenate([-AiT, ArT], axis=1)
    D2r = np.zeros((128, 128), np.float32)
    D2i = np.zeros((128, 128), np.float32)
    D2r[:64, :64] = C2r
    D2r[64:, 64:] = C2r
    D2i[:64, :64] = C2i
    D2i[64:, 64:] = C2i
    return np.stack([D1r, D1i, D2r, D2i])


def build_program(reps: int = 1, split_waits: bool = True,
                  sp_loads: bool = False, sp_stores: bool = False,
                  gp_stores: bool = False, load_mode: str = "l1",
                  probe_contig_loads: bool = False,
                  probe_contig_stores: bool = False,
                  probe_no_compute: bool = False,
                  probe_no_dma: bool = False,
                  no_tilepos: bool = False,
                  deep_bufs: bool = False):
    """reps > 1 unrolls the whole pipeline in-NEFF over the same data so the
    marginal cost per rep can be measured without the ~65ms axon dispatch
    overhead."""
    nc = bass.Bass("TRN2", target_bir_lowering=False, debug=False)
    x = nc.dram_tensor("x", [BPC, C2, H, W], F32, kind="ExternalInput").ap()
    dm = nc.dram_tensor("dmats", [4, 128, 128], F32, kind="ExternalInput").ap()
    y = nc.dram_tensor("y", [BPC, C2, HP, WP], F32, kind="ExternalOutput").ap()

    with TileContext(nc) as tc:
        with (
            tc.tile_pool(name="consts", bufs=1) as cpool,
            tc.tile_pool(name="inp", bufs=3 if deep_bufs else 2) as ipool,
            tc.tile_pool(name="sb1", bufs=6 if deep_bufs else 4) as s1pool,
            tc.tile_pool(name="sbout", bufs=3 if deep_bufs else 2) as opool,
            tc.tile_pool(name="ps1", bufs=4, space="PSUM") as p1pool,
            tc.tile_pool(name="ps2", bufs=4 if deep_bufs else 2,
                         space="PSUM") as p2pool,
        ):
            dmf = cpool.tile([128, 512], F32, tag="dmf")
            dmb = cpool.tile([128, 512], BF16, tag="dmb")
            for k in range(4):
                nc.sync.dma_start(out=dmf[:, 128 * k : 128 * (k + 1)], in_=dm[k])
            nc.vector.tensor_copy(out=dmb, in_=dmf)
            d1rb = dmb[:, 0:128]
            d1ib = dmb[:, 128:256]
            d2rb = dmb[:, 256:384]
            d2ib = dmb[:, 384:512]

            for b in [b for _ in range(reps) for b in range(BPC)]:
                # SBUF input layout: partitions = (channel parity, h); free =
                # (slot, w) where slot s holds pair 4*(s//4) + {0,2,1,3}[s%4]
                # so each quad (o, qp) is a contiguous 128-col lhsT slice.
                in_r = ipool.tile([128, (C2 // 4) * W], BF16, tag="in_r")
                in_i = ipool.tile([128, (C2 // 4) * W], BF16, tag="in_i")
                # channel = 8o + 4bb + 2rr + two ; pair = 4o + 2bb + rr
                # slot 4o + 2rr + bb holds pair 4o + 2bb + rr. Loads for
                # rr=0 (needed by the qp=0 quads) are issued first so
                # compute can start after half the batch has landed.
                if probe_no_dma:
                    pass
                elif probe_contig_loads:
                    # timing probe only: plain contiguous cast-loads
                    nc.gpsimd.dma_start(
                        out=in_r,
                        in_=x[b, : C2 // 2].rearrange(
                            "(pp c) h w -> pp (c h w)", pp=128
                        ),
                    )
                    nc.gpsimd.dma_start(
                        out=in_i,
                        in_=x[b, C2 // 2 :].rearrange(
                            "(pp c) h w -> pp (c h w)", pp=128
                        ),
                    )
                for rr in (() if (probe_contig_loads or probe_no_dma) else range(2)):
                    for tile, lohalf in ((in_r, x[b, : C2 // 2]),
                                         (in_i, x[b, C2 // 2 :])):
                        src = lohalf.rearrange(
                            "(o bb rr two) h w -> o bb rr two h w",
                            o=16, bb=2, rr=2, two=2,
                        )
                        if load_mode == "l2":
                            # slot 32rr + 2o + bb: one contiguous-dst DMA/rr
                            nc.gpsimd.dma_start(
                                out=tile[:, 2048 * rr : 2048 * rr + 2048],
                                # (o,bb,two,h,w) -> (two,h,o,bb,w)
                                in_=src[:, :, rr].transpose([2, 3, 0, 1, 4]),
                                single_packet=sp_loads,
                            )
                            continue
                        dst = tile.rearrange(
                            "p (o sw) -> p o sw", o=16, sw=256
                        )
                        for bb in range(2):
                            so = 64 * (2 * rr + bb)
                            nc.gpsimd.dma_start(
                                out=dst[:, :, so : so + 64],
                                # (o,two,h,w) -> (two,h,o,w)
                                in_=src[:, bb, rr].transpose([1, 2, 0, 3]),
                                single_packet=sp_loads,
                            )

                sb_out = opool.tile([128, 2048], F32, tag="sb_out")
                if probe_no_compute:
                    nc.vector.memset(sb_out, 0.0)
                if probe_no_dma:
                    nc.gpsimd.memset(in_r[:, :], 0.0)
                    nc.gpsimd.memset(in_i[:, :], 0.0)
                for q in (() if probe_no_compute else range(C2 // 8)):
                    # quad (o, qp): complex channels {8o+2qp, +1, +4, +5}
                    # qp-major order: all rr=0-fed quads first
                    qp, o = q // 16, q % 16
                    if load_mode == "l2":
                        lo = 2048 * qp + 128 * o
                    else:
                        lo = 256 * o + 128 * qp
                    psum1 = p1pool.tile([128, 128], F32, tag="ps1")
                    nc.tensor.matmul(
                        out=psum1,
                        lhsT=in_r[:, lo : lo + 128],
                        rhs=d1rb,
                        start=True,
                        stop=False,
                        tile_position=None if no_tilepos else (0, 0),
                    )
                    nc.tensor.matmul(
                        out=psum1,
                        lhsT=in_i[:, lo : lo + 128],
                        rhs=d1ib,
                        start=False,
                        stop=True,
                        tile_position=None if no_tilepos else (0, 0),
                    )
                    sb1 = s1pool.tile([128, 128], BF16, tag="sb1")
                    nc.vector.tensor_copy(out=sb1, in_=psum1)

                    psum2 = p2pool.tile([128, 128], F32, tag="ps2")
                    cb = 64 * qp
                    nc.tensor.matmul(
                        out=psum2[cb : cb + 64, :],
                        lhsT=sb1[:, 0:64],
                        rhs=d2rb,
                        start=True,
                        stop=False,
                        tile_position=None if no_tilepos else (0, cb),
                    )
                    nc.tensor.matmul(
                        out=psum2[cb : cb + 64, :],
                        lhsT=sb1[:, 64:128],
                        rhs=d2ib,
                        start=False,
                        stop=True,
                        tile_position=None if no_tilepos else (0, cb),
                    )
                    nc.scalar.copy(
                        out=sb_out[cb : cb + 64, 128 * o : 128 * (o + 1)],
                        in_=psum2[cb : cb + 64, :],
                    )
                # channel = 128*ri + 8*o + 4*t + 2*qp + s ; partitions (qp s h)
                sbv = sb_out.rearrange(
                    "p (o t ri w) -> p (o t) ri w", o=16, t=2, ri=2, w=WP
                )
                st_eng = nc.gpsimd if gp_stores else nc.sync
                if probe_no_dma:
                    continue
                if probe_contig_stores:
                    st_eng.dma_start(
                        out=y[b].rearrange("(pp c) h w -> pp (c h w)", pp=128),
                        in_=sb_out,
                    )
                for ri in (() if probe_contig_stores else range(2)):
                    st_eng.dma_start(
                        out=y[b, 128 * ri : 128 * (ri + 1)].rearrange(
                            "(o t qp s) h w -> (qp s h) (o t) w",
                            o=16, t=2, qp=2, s=2,
                        ),
                        in_=sbv[:, :, ri, :],
                        single_packet=sp_stores,
                    )
    if split_waits:
        _split_multi_waits(nc)
    return nc


_CACHED = {}


def _get_program():
    if "nc" not in _CACHED:
        _CACHED["nc"] = build_program()
        _CACHED["consts"] = _dft_constants()
    return _CACHED["nc"], _CACHED["consts"]


def kernel(x: np.ndarray) -> np.ndarray:
    assert x.shape == (B_FULL, C2, H, W) and x.dtype == np.float32
    nc, dmats = _get_program()
    x = np.ascontiguousarray(x)
    in_maps = [
        {"x": x[BPC * k : BPC * (k + 1)], "dmats": dmats}
        for k in range(N_CORES)
    ]
    res = run_bass_kernel_spmd(nc, in_maps, list(range(N_CORES)))
    out = np.concatenate(
        [res.results[k]["y"] for k in range(N_CORES)], axis=0
    )
    return out.astype(np.float32, copy=False)


if __name__ == "__main__":
    rng = np.random.default_rng(0)
    x = rng.standard_normal((B_FULL, C2, H, W)).astype(np.float32)
    y = kernel(x)
    print("kernel output", y.shape, y.dtype)
